# revision 1
# baseline (speedup 1.0000x reference)
"""Trainium2 Bass kernel for nn_DHSLayer (DHS-pruned ViT attention layer).

Strategy: data-parallel over batch (B=128 -> 16 per core x 8 cores).
All matmuls in fp16 (fp32 PSUM accumulation); softmax without max-subtraction
(scores are O(1) here); denominator folded into the ctx matmul via a ones
column appended to V; residual folded into the dense weight (We = Wd + I);
V bias folded into the dense bias (softmax rows sum to 1). Transposes done
by DMA xbar (fp16) instead of the PE.

Self-contained: hardcodes shapes B=128, S=197, D=768, H=12, DH=64.
"""

import os
import sys
from contextlib import ExitStack

import numpy as np

for _p in ("/opt/trn_rl_repo", "/root/.axon_site/_ro/trn_rl_repo"):
    if os.path.isdir(_p) and _p not in sys.path:
        sys.path.append(_p)

import concourse.bass as bass
import concourse.tile as tile
from concourse import bacc, mybir
from concourse import bass_utils
from concourse.masks import make_identity

F16 = mybir.dt.float16
F32 = mybir.dt.float32
AF = mybir.ActivationFunctionType
ALU = mybir.AluOpType

D = 768
S = 197
S2 = 2 * S
NH = 12
DH = 64
NCORES = 8
DT = 6  # number of 128-wide d tiles
TT = ((0, 128), (128, 69))  # token tiles covering S=197
TP = (128, 80)  # padded (multiple-of-16) row counts for DMA transpose
SP_ = 208  # padded per-dtile stride in transposed (feature-major) layouts
KS = 256  # per-dtile stride in k16 (scores lhsT over-read to M=128)
# sigmoid(z) >= 0.05  <=>  z >= log(0.05/0.95)
LOGIT_THR = -2.9444389791664403


def _body(ctx, tc, io, n_b):
    nc = tc.nc
    const = ctx.enter_context(tc.tile_pool(name="const", bufs=1))
    p_in = ctx.enter_context(tc.tile_pool(name="p_in", bufs=5))
    p_x16 = ctx.enter_context(tc.tile_pool(name="p_x16", bufs=3))
    p_qk = ctx.enter_context(tc.tile_pool(name="p_qk", bufs=2))
    p_v = ctx.enter_context(tc.tile_pool(name="p_v", bufs=4))
    p_mlp = ctx.enter_context(tc.tile_pool(name="p_mlp", bufs=2))
    p_att = ctx.enter_context(tc.tile_pool(name="p_att", bufs=4))
    p_ctx = ctx.enter_context(tc.tile_pool(name="p_ctx", bufs=3))
    p_sel = ctx.enter_context(tc.tile_pool(name="p_sel", bufs=3))
    ps = ctx.enter_context(tc.tile_pool(name="ps", bufs=8, space="PSUM"))

    def pst(shape, dtype=F32):
        return ps.tile(shape, dtype, tag="ps", name="pst")

    pp = ptp = psp = pcp = pd = pst
    ptp = lambda shape, dtype=F16: pst(shape, dtype)

    # ---- first input loads (before the bulky weight DMAs) ----
    def load_pair(p):
        st = {"p": p}
        for b01 in range(2):
            xa = p_in.tile([128, D], F32, tag="x32a", name="x32a")
            nc.sync.dma_start(xa, io["hs"][2 * p + b01, 0:128, :])
            st[f"x32a{b01}"] = xa
        for b01 in range(2):
            xb = p_in.tile([69, D], F32, tag="x32b", name="x32b")
            nc.sync.dma_start(xb, io["hs"][2 * p + b01, 128:197, :])
            st[f"x32b{b01}"] = xb
        return st

    st0 = load_pair(0)

    # ---- constants ----
    ones16 = const.tile([1, 1], F16)
    nc.vector.memset(ones16, 1.0)
    ident16 = const.tile([128, 128], F16)
    nc.sync.dma_start(ident16, io["ident"])

    wq16 = const.tile([128, DT * D], F16)
    for _j in range(DT):
        nc.sync.dma_start(
            wq16[:, _j * D : (_j + 1) * D], io["wq"][_j * 128 : (_j + 1) * 128, :]
        )
    wk16 = const.tile([128, DT * D], F16)
    for _j in range(DT):
        nc.sync.dma_start(
            wk16[:, _j * D : (_j + 1) * D], io["wk"][_j * 128 : (_j + 1) * 128, :]
        )
    wv16 = const.tile([128, DT * D], F16)
    for _j in range(DT):
        nc.sync.dma_start(
            wv16[:, _j * D : (_j + 1) * D], io["wv"][_j * 128 : (_j + 1) * 128, :]
        )
    w116 = const.tile([128, DT * 64], F16)
    nc.sync.dma_start(w116.rearrange("p (j n) -> p j n", j=DT), io["w1"].rearrange("(j p) n -> p j n", p=128))
    w216 = const.tile([64, 1], F16)
    nc.sync.dma_start(w216, io["w2"])

    bqs32 = const.tile([128, DT], F32)
    nc.sync.dma_start(bqs32, io["bqs"])
    bks32 = const.tile([128, DT], F32)
    nc.sync.dma_start(bks32, io["bks"])
    b1c32 = const.tile([64, 1], F32)
    nc.sync.dma_start(b1c32, io["b1c"])
    bde32 = const.tile([128, D], F32)
    bde_bcast = bass.AP(
        tensor=io["bde"].tensor,
        offset=io["bde"].offset,
        ap=[[0, 128], [1, D]],
    )
    nc.sync.dma_start(bde32, bde_bcast)

    we16 = const.tile([128, DT * D], F16)
    for _j in range(DT):
        nc.sync.dma_start(
            we16[:, _j * D : (_j + 1) * D], io["we"][_j * 128 : (_j + 1) * 128, :]
        )

    out = io["out"]

    # Layout strides for pair tiles
    SPP = 2 * SP_        # 416: xfm16/ctxf16 per-dtile stride (2 batches x 208)
    QS = 2 * S           # 394: q16 per-dtile stride
    KSP = 2 * S + 59     # 453: k16 per-dtile stride (197 b0 | 197 b1 | 59 pad)

    def emit_xpose(st):
        # cast to fp16 then PE-transpose into the pair feature-major tile;
        # transposes of (b0, b1) at the same token-tile share one PSUM tile
        # and a single strided DVE copy.
        x16 = []
        for b01 in range(2):
            xa = p_x16.tile([128, D], F16, tag="x16a", name="x16a")
            nc.vector.tensor_copy(xa, st[f"x32a{b01}"])
            xb = p_x16.tile([69, D], F16, tag="x16b", name="x16b")
            nc.vector.tensor_copy(xb, st[f"x32b{b01}"])
            x16.append((xa, xb))
        xfm16 = p_x16.tile([128, DT * SPP], F16, name="xfm16")
        for ti, (t0, tsz) in enumerate(TT):
            for j in range(DT):
                tp = pst([128, 2, 128], F16)
                for b01 in range(2):
                    nc.tensor.transpose(
                        tp[:, b01, 0:tsz],
                        x16[b01][ti][0:tsz, j * 128 : (j + 1) * 128],
                        ident16[0:tsz, 0:tsz],
                    )
                dst = xfm16[:, j * SPP + t0 :]
                dst = bass.AP(
                    tensor=dst.tensor,
                    offset=dst.offset,
                    ap=[list(dst.ap[0]), [SP_, 2], [1, tsz]],
                )
                nc.vector.tensor_copy(dst, tp[:, :, 0:tsz])
        st["xfm16"] = xfm16

    def emit_qk_group(st, jo):
        xfm16 = st["xfm16"]
        if jo == 0:
            st["q16"] = p_qk.tile([128, DT * QS], F16, name="q16")
            st["k16"] = p_qk.tile([128, DT * KSP], F16, name="k16")
            nc.vector.memset(
                st["k16"].rearrange("p (j n) -> p j n", j=DT)[:, :, 2 * S : KSP], 0.0
            )
        rhs = xfm16[:, jo * 0 :]  # placeholder (unused)
        qp = pst([128, QS])
        for ji in range(DT):
            r = xfm16[:, ji * SPP :]
            r3 = bass.AP(
                tensor=r.tensor,
                offset=r.offset,
                ap=[list(r.ap[0]), [SP_, 2], [1, S]],
            )
            nc.tensor.matmul(
                qp.rearrange("p (s x) -> p s x", s=2),
                lhsT=wq16[:, ji * D + jo * 128 : ji * D + (jo + 1) * 128],
                rhs=r3,
                start=(ji == 0),
                stop=(ji == DT - 1),
            )
        nc.vector.tensor_scalar(
            st["q16"][:, jo * QS : (jo + 1) * QS], qp, 0.125,
            bqs32[:, jo : jo + 1], op0=ALU.mult, op1=ALU.add,
        )
        kp = pst([128, QS])
        for ji in range(DT):
            r = xfm16[:, ji * SPP :]
            r3 = bass.AP(
                tensor=r.tensor,
                offset=r.offset,
                ap=[list(r.ap[0]), [SP_, 2], [1, S]],
            )
            nc.tensor.matmul(
                kp.rearrange("p (s x) -> p s x", s=2),
                lhsT=wk16[:, ji * D + jo * 128 : ji * D + (jo + 1) * 128],
                rhs=r3,
                start=(ji == 0),
                stop=(ji == DT - 1),
            )
        nc.vector.tensor_scalar(
            st["k16"].rearrange("p (j n) -> p j n", j=DT)[:, jo, 0 : 2 * S],
            kp,
            1.0,
            bks32[:, jo : jo + 1],
            op0=ALU.mult,
            op1=ALU.add,
        )

    def emit_v(st, b01, ti, half):
        xfm16 = st["xfm16"]
        if ti == 0 and half == 0:
            st[f"va{b01}"] = p_v.tile([128, NH, DH + 1], F16, tag="va", name="va")
            st[f"vb{b01}"] = p_v.tile([69, NH, DH + 1], F16, tag="vb", name="vb")
            nc.vector.memset(st[f"va{b01}"][:, :, DH : DH + 1], 1.0)
            nc.vector.memset(st[f"vb{b01}"][:, :, DH : DH + 1], 1.0)
        t0, tsz = TT[ti]
        vt = st[f"va{b01}"] if ti == 0 else st[f"vb{b01}"]
        vp = pst([128, 384])
        for ji in range(DT):
            nc.tensor.matmul(
                vp[0:tsz, :],
                lhsT=xfm16[
                    :, ji * SPP + b01 * SP_ + t0 : ji * SPP + b01 * SP_ + t0 + tsz
                ],
                rhs=wv16[:, ji * D + half * 384 : ji * D + (half + 1) * 384],
                start=(ji == 0),
                stop=(ji == DT - 1),
            )
        nc.scalar.copy(
            vt[0:tsz, half * 6 : (half + 1) * 6, 0:DH],
            vp[0:tsz, :].rearrange("p (h d) -> p h d", h=6),
        )

    def emit_mlp(st):
        xfm16 = st["xfm16"]
        hp_ = pst([64, QS])
        for ji in range(DT):
            r = xfm16[:, ji * SPP :]
            r3 = bass.AP(
                tensor=r.tensor,
                offset=r.offset,
                ap=[list(r.ap[0]), [SP_, 2], [1, S]],
            )
            nc.tensor.matmul(
                hp_.rearrange("p (s x) -> p s x", s=2),
                lhsT=w116[:, ji * 64 : (ji + 1) * 64],
                rhs=r3,
                start=(ji == 0),
                stop=(ji == DT - 1),
            )
        h116 = p_mlp.tile([64, QS], F16, name="h116")
        nc.scalar.activation(h116, hp_, AF.Relu, bias=b1c32, scale=1.0)
        lp = pst([1, QS])
        nc.tensor.matmul(lp, lhsT=w216, rhs=h116, start=True, stop=True)
        m16 = p_mlp.tile([1, QS], F16, name="m16")
        nc.vector.tensor_scalar(m16, lp, float(io["thr"]), None, op0=ALU.is_ge)
        nc.vector.memset(m16[0:1, 0:1], 1.0)  # CLS b0
        nc.vector.memset(m16[0:1, S : S + 1], 1.0)  # CLS b1
        st["m16"] = m16

    def emit_mcols(st):
        for b01 in range(2):
            mca = p_sel.tile([128, 1], F32, tag="mca", name="mca")
            mcb = p_sel.tile([69, 1], F32, tag="mcb", name="mcb")
            for (t0, tsz), mc in zip(TT, (mca, mcb)):
                mp = pst([tsz, 1])
                nc.tensor.matmul(
                    mp,
                    lhsT=st["m16"][0:1, b01 * S + t0 : b01 * S + t0 + tsz],
                    rhs=ones16,
                    start=True,
                    stop=True,
                )
                nc.vector.tensor_copy(mc[0:tsz, :], mp)
            st[f"mca{b01}"] = mca
            st[f"mcb{b01}"] = mcb

    def emit_scores(st, b01, hp):
        if hp == 0:
            st[f"ca{b01}"] = p_ctx.tile([128, D], F16, tag="ca", name="ca")
            st[f"cb{b01}"] = p_ctx.tile([69, D], F16, tag="cb", name="cb")
        q16, k16 = st["q16"], st["k16"]
        sps = [pst([128, S2]), pst([128, S2])]
        for i in range(2):
            for sub in range(2):
                pb = sub * 64
                nc.tensor.matmul(
                    sps[sub][:, i * S : (i + 1) * S],
                    lhsT=k16[
                        pb : pb + 64,
                        hp * KSP + b01 * S + i * 128 : hp * KSP + b01 * S + (i + 1) * 128,
                    ],
                    rhs=q16[pb : pb + 64, hp * QS + b01 * S : hp * QS + (b01 + 1) * S],
                    start=True,
                    stop=True,
                )
        exps = []
        for sub in range(2):
            expt = p_att.tile([128, S2], F16, tag="expt", name="expt", bufs=14)
            nc.scalar.activation(expt, sps[sub], AF.Exp)
            exps.append(expt)
        st.setdefault("exps", {})[(b01, hp)] = exps

    def emit_ctx(st, b01, hp):
        exps = st["exps"].pop((b01, hp))
        cp4 = pst([128, 4 * (DH + 1)])
        vts = (st[f"va{b01}"], st[f"vb{b01}"])
        for sub in range(2):
            h = hp * 2 + sub
            for qi, (q0, qsz) in enumerate(TT):
                c0 = (qi * 2 + sub) * (DH + 1)
                for i, (kt0, ksz) in enumerate(TT):
                    nc.tensor.matmul(
                        cp4[0:qsz, c0 : c0 + DH + 1],
                        lhsT=exps[sub][0:ksz, i * S + q0 : i * S + q0 + qsz],
                        rhs=vts[i][0:ksz, h, :],
                        start=(i == 0),
                        stop=(i == 1),
                    )
        for qi, ((q0, qsz), ct) in enumerate(
            zip(TT, (st[f"ca{b01}"], st[f"cb{b01}"]))
        ):
            half = cp4[0:qsz, qi * 130 : (qi + 1) * 130].rearrange(
                "p (s x) -> p s x", s=2
            )
            rc2 = p_att.tile([128, 2], F32, tag="rc", name="rc2", bufs=6)
            nc.vector.reciprocal(rc2[0:qsz, :], half[:, :, DH : DH + 1])
            rc2s = rc2[0:qsz, 0:2]
            rc2b = bass.AP(
                tensor=rc2s.tensor,
                offset=rc2s.offset,
                ap=[list(rc2s.ap[0]), [1, 2], [0, DH]],
            )
            nc.vector.tensor_mul(
                ct[0:qsz, hp * 128 : (hp + 1) * 128].rearrange(
                    "p (s x) -> p s x", s=2
                ),
                half[:, :, 0:DH],
                rc2b,
            )

    def emit_ctxT(st, js):
        if "ctxf16" not in st:
            st["ctxf16"] = p_ctx.tile([128, DT * SPP], F16, name="ctxf16")
        ctxf16 = st["ctxf16"]
        for j in js:
            for ti, (t0, tsz) in enumerate(TT):
                tp2 = pst([128, 2, 128], F16)
                for b01 in range(2):
                    ct = st[f"ca{b01}"] if ti == 0 else st[f"cb{b01}"]
                    nc.tensor.transpose(
                        tp2[:, b01, 0:tsz],
                        ct[0:tsz, j * 128 : (j + 1) * 128],
                        ident16[0:tsz, 0:tsz],
                    )
                dst = ctxf16[:, j * SPP + t0 :]
                dst = bass.AP(
                    tensor=dst.tensor,
                    offset=dst.offset,
                    ap=[list(dst.ap[0]), [SP_, 2], [1, tsz]],
                )
                nc.vector.tensor_copy(dst, tp2[:, :, 0:tsz])

    def emit_dense(st, b01):
        b = 2 * st["p"] + b01
        ctxf16 = st["ctxf16"]
        for (t0, tsz), x32, mc in zip(
            TT,
            (st[f"x32a{b01}"], st[f"x32b{b01}"]),
            (st[f"mca{b01}"], st[f"mcb{b01}"]),
        ):
            om = p_sel.tile([128, 1], F32, tag="om", name="om")
            nc.vector.tensor_scalar(
                om[0:tsz, :], mc[0:tsz, :], -1.0, 1.0, op0=ALU.mult, op1=ALU.add
            )
            t1 = p_sel.tile([128, D], F32, tag="t1", name="t1")
            nc.vector.tensor_scalar(
                t1[0:tsz, :], x32[0:tsz, :], om[0:tsz, 0:1], None, op0=ALU.mult
            )
            z = p_sel.tile([128, D], F32, tag="z", name="z")
            nc.vector.scalar_tensor_tensor(
                z[0:tsz, :], bde32[0:tsz, :], mc[0:tsz, 0:1], t1[0:tsz, :],
                op0=ALU.mult, op1=ALU.add,
            )
            o32 = p_sel.tile([128, D], F32, tag="o32", name="o32")
            for half in range(2):
                ap_ = pst([128, 384])
                for ji in range(DT):
                    nc.tensor.matmul(
                        ap_[0:tsz, :],
                        lhsT=ctxf16[
                            :,
                            ji * SPP + b01 * SP_ + t0 : ji * SPP + b01 * SP_ + t0 + tsz,
                        ],
                        rhs=we16[:, ji * D + half * 384 : ji * D + (half + 1) * 384],
                        start=(ji == 0),
                        stop=(ji == DT - 1),
                    )
                nc.vector.scalar_tensor_tensor(
                    o32[0:tsz, half * 384 : (half + 1) * 384],
                    ap_[0:tsz, :],
                    mc[0:tsz, 0:1],
                    z[0:tsz, half * 384 : (half + 1) * 384],
                    op0=ALU.mult,
                    op1=ALU.add,
                )
            nc.sync.dma_start(out[b, t0 : t0 + tsz, :], o32[0:tsz, :])

    # ---- main pair pipeline: proj(pair p) interleaved with attention(p-1) ----
    assert n_b % 2 == 0
    npair = n_b // 2
    prev = None
    for p in range(npair):
        st = st0 if p == 0 else load_pair(p)
        emit_xpose(st)
        if prev is not None:
            emit_mcols(prev)
        for hp in range(DT):
            if prev is not None:
                emit_scores(prev, 0, hp)
                emit_scores(prev, 1, hp)
            emit_qk_group(st, hp)
            if prev is not None and hp > 0:
                emit_ctx(prev, 0, hp - 1)
                emit_ctx(prev, 1, hp - 1)
        if prev is not None:
            emit_ctx(prev, 0, DT - 1)
            emit_ctx(prev, 1, DT - 1)
        emit_v(st, 0, 0, 0)
        if prev is not None:
            emit_ctxT(prev, range(0, 3))
        emit_v(st, 0, 0, 1)
        if prev is not None:
            emit_ctxT(prev, range(3, 6))
        emit_v(st, 0, 1, 0)
        emit_v(st, 0, 1, 1)
        if prev is not None:
            emit_dense(prev, 0)
        emit_v(st, 1, 0, 0)
        emit_v(st, 1, 0, 1)
        emit_v(st, 1, 1, 0)
        emit_v(st, 1, 1, 1)
        emit_mlp(st)
        if prev is not None:
            emit_dense(prev, 1)
        prev = st

    # drain: attention + dense of the last pair
    st = prev
    emit_mcols(st)
    for hp in range(DT):
        emit_scores(st, 0, hp)
        emit_scores(st, 1, hp)
        if hp > 0:
            emit_ctx(st, 0, hp - 1)
            emit_ctx(st, 1, hp - 1)
    emit_ctx(st, 0, DT - 1)
    emit_ctx(st, 1, DT - 1)
    emit_ctxT(st, range(DT))
    emit_dense(st, 0)
    emit_dense(st, 1)


def build_nc(n_b, thr):
    nc = bacc.Bacc(
        "TRN2", target_bir_lowering=False, debug=False, num_devices=NCORES
    )
    io = {
        "hs": nc.dram_tensor("hs", [n_b, S, D], F32, kind="ExternalInput").ap(),
        "wq": nc.dram_tensor("wq", [D, D], F16, kind="ExternalInput").ap(),
        "wk": nc.dram_tensor("wk", [D, D], F16, kind="ExternalInput").ap(),
        "wv": nc.dram_tensor("wv", [D, D], F16, kind="ExternalInput").ap(),
        "we": nc.dram_tensor("we", [D, D], F16, kind="ExternalInput").ap(),
        "w1": nc.dram_tensor("w1", [D, 64], F16, kind="ExternalInput").ap(),
        "w2": nc.dram_tensor("w2", [64, 1], F16, kind="ExternalInput").ap(),
        "bqs": nc.dram_tensor("bqs", [128, DT], F32, kind="ExternalInput").ap(),
        "bks": nc.dram_tensor("bks", [128, DT], F32, kind="ExternalInput").ap(),
        "b1c": nc.dram_tensor("b1c", [64, 1], F32, kind="ExternalInput").ap(),
        "bde": nc.dram_tensor("bde", [D], F32, kind="ExternalInput").ap(),
        "ident": nc.dram_tensor("ident", [128, 128], F16, kind="ExternalInput").ap(),
        "out": nc.dram_tensor("out", [n_b, S, D], F32, kind="ExternalOutput").ap(),
        "thr": thr,
    }
    with tile.TileContext(nc) as tc, ExitStack() as ctx:
        _body(ctx, tc, io, n_b)
    nc.compile()
    return nc


def make_host_inputs(Wq, bq, Wk, bk, Wv, bv, Wd, bd, W1, b1, W2, b2):
    """Host-side weight prep shared by all cores."""
    f32 = np.float32
    Wd = np.asarray(Wd, f32)
    bv = np.asarray(bv, f32)
    bd = np.asarray(bd, f32)
    we = Wd + np.eye(D, dtype=f32)
    bde = (bv @ we + bd).astype(f32)
    return {
        "wq": np.ascontiguousarray(np.asarray(Wq, f32).astype(np.float16)),
        "wk": np.ascontiguousarray(np.asarray(Wk, f32).astype(np.float16)),
        "wv": np.ascontiguousarray(np.asarray(Wv, f32).astype(np.float16)),
        "we": np.ascontiguousarray(we.astype(np.float16)),
        "w1": np.ascontiguousarray(np.asarray(W1, f32).astype(np.float16)),
        "w2": np.ascontiguousarray(
            np.asarray(W2, f32).astype(np.float16).reshape(64, 1)
        ),
        "bqs": np.ascontiguousarray(
            (np.asarray(bq, f32) / 8.0).reshape(DT, 128).T
        ),
        "bks": np.ascontiguousarray(np.asarray(bk, f32).reshape(DT, 128).T),
        "b1c": np.ascontiguousarray(np.asarray(b1, f32).reshape(64, 1)),
        "bde": bde,
        "ident": np.eye(128, dtype=np.float16),
    }, float(LOGIT_THR - float(np.asarray(b2, f32).reshape(-1)[0]))


_NC_CACHE = {}


def kernel(hidden_states, Wq, bq, Wk, bk, Wv, bv, Wd, bd, W1, b1, W2, b2):
    hs = np.ascontiguousarray(np.asarray(hidden_states, np.float32))
    B = hs.shape[0]
    n_b = B // NCORES
    weights, thr = make_host_inputs(Wq, bq, Wk, bk, Wv, bv, Wd, bd, W1, b1, W2, b2)

    key = (n_b, thr)
    if key not in _NC_CACHE:
        _NC_CACHE[key] = build_nc(n_b, thr)
    nc = _NC_CACHE[key]

    in_maps = [
        {**weights, "hs": np.ascontiguousarray(hs[c * n_b : (c + 1) * n_b])}
        for c in range(NCORES)
    ]
    res = bass_utils.run_bass_kernel_spmd(nc, in_maps, core_ids=list(range(NCORES)))
    return np.concatenate(
        [res.results[c]["out"] for c in range(NCORES)], axis=0
    ).astype(np.float32)

